# revision 16
# baseline (speedup 1.0000x reference)
"""Trainium2 Bass kernel for the AttentionAggregator GNN message-passing module.

Reference computation (per node i):
    scores over M=16384 candidate columns, masked to the <=10 sampled
    neighbor columns (neigh_idx[i, :]), softmax, then weighted sum of the
    neighbor embeddings.

Because the additive mask kills every column except the <=10 sampled ones,
the full [N, M] score matrix never needs to be materialized: per node we
only need the 10 dot products  f_i . e_{neigh(i,s)},  a softmax over the
*unique* sampled columns (duplicate columns within a row count once), and
the weighted sum of those embedding rows.

Sharding: node batch dim N=4096 is split across 8 cores (512 nodes each).
The feature table stays in DRAM (replicated); each core gathers only the
rows it needs via indirect DMA, then computes scores / softmax / the
weighted sum on DVE+ACT per 128-node tile.

Gather strategy is chosen AT RUNTIME by a tiny self-test, because the
SWDGE multi-index indirect-DMA path is flaky across device/worker states:
  - fast path: one indirect DMA per tile gathers all 10 neighbor rows
    ([128, 10] offset AP + 3D dest so the ucode pairs one 256-elem chunk
    per index). ~10us of Pool time per core.
  - safe path: one indirect DMA per (tile, sample) with [128, 1] offset
    APs (the only pattern that survives degraded workers). ~46us Pool.
The self-test gathers known rows from a small table on core 0 and checks
them host-side; the first mismatch permanently selects the safe path.
"""

import numpy as np

import concourse.bass as bass
import concourse.mybir as mybir
from concourse import bacc, tile
from concourse import bass_utils

# Problem constants (hardcoded per the harness contract).
V, FDIM = 100000, 256
N, S = 4096, 10
NCORES = 8
NPC = N // NCORES          # 512 nodes per core
P = 128                    # SBUF partitions
NTILES = NPC // P          # 4 node-tiles per core
NEG = np.float32(-1.0e30)  # additive mask for duplicate sample slots

_CACHE = {}


def _build_nc(multi_idx_gather):
    nc = bacc.Bacc("TRN2", target_bir_lowering=False, debug=False,
                   num_devices=NCORES)
    f32 = mybir.dt.float32
    i32 = mybir.dt.int32

    features = nc.dram_tensor("features", [V, FDIM], f32, kind="ExternalInput").ap()
    node_rows = nc.dram_tensor("node_rows", [NPC, 1], i32, kind="ExternalInput").ap()
    neigh_rows = nc.dram_tensor("neigh_rows", [NPC, S], i32, kind="ExternalInput").ap()
    dup_mask = nc.dram_tensor("dup_mask", [NPC, S], f32, kind="ExternalInput").ap()
    out = nc.dram_tensor("out", [NPC, FDIM], f32, kind="ExternalOutput").ap()

    with tile.TileContext(nc) as tc:
        with tc.tile_pool(name="idx", bufs=NTILES) as idx_pool, \
             tc.tile_pool(name="emb", bufs=3) as emb_pool, \
             tc.tile_pool(name="sm", bufs=NTILES) as sm_pool, \
             tc.tile_pool(name="acc", bufs=3) as acc_pool:
            for t in range(NTILES):
                rows = slice(t * P, (t + 1) * P)

                nidx = idx_pool.tile([P, 1], i32, tag="nidx")
                nc.sync.dma_start(out=nidx[:], in_=node_rows[rows, :])
                eidx = idx_pool.tile([P, S], i32, tag="eidx")
                nc.sync.dma_start(out=eidx[:], in_=neigh_rows[rows, :])
                mask = idx_pool.tile([P, S], f32, tag="mask")
                nc.sync.dma_start(out=mask[:], in_=dup_mask[rows, :])

                # Gather this tile's node feature rows: ftile[p] = features[nidx[p]]
                ftile = emb_pool.tile([P, FDIM], f32, tag="ftile")
                nc.gpsimd.indirect_dma_start(
                    out=ftile[:], out_offset=None,
                    in_=features,
                    in_offset=bass.IndirectOffsetOnAxis(ap=nidx[:, :1], axis=0),
                )
                # Gather the 10 neighbor embedding rows per node, laid out
                # per-partition as 10 concatenated rows of 256.
                etile = emb_pool.tile([P, S * FDIM], f32, tag="etile")
                if multi_idx_gather:
                    # One indirect DMA; 3D dest AP so the ucode pairs one
                    # 256-elem chunk with each of the 1280 indices (a flat
                    # 2D dest mispairs chunks and indices).
                    nc.gpsimd.indirect_dma_start(
                        out=etile[:].rearrange("p (s f) -> p s f", s=S),
                        out_offset=None,
                        in_=features,
                        in_offset=bass.IndirectOffsetOnAxis(ap=eidx[:, :], axis=0),
                    )
                else:
                    # Safe path: one [128, 1]-offset indirect DMA per sample.
                    for s in range(S):
                        nc.gpsimd.indirect_dma_start(
                            out=etile[:, s * FDIM:(s + 1) * FDIM], out_offset=None,
                            in_=features,
                            in_offset=bass.IndirectOffsetOnAxis(
                                ap=eidx[:, s:s + 1], axis=0),
                        )

                # scores[p, s] = sum_d ftile[p, d] * etile[p, s*F + d]
                # (fused multiply+row-reduce on DVE; tensor_tensor_reduce is
                # broken on HW in this environment, scalar_tensor_tensor with
                # accum_out does the same thing.)
                scores = sm_pool.tile([P, S], f32, tag="scores")
                scratch = acc_pool.tile([P, FDIM], f32, tag="scratch")
                for s in range(S):
                    nc.vector.scalar_tensor_tensor(
                        out=scratch[:],
                        in0=ftile[:],
                        scalar=0.0,
                        in1=etile[:, s * FDIM:(s + 1) * FDIM],
                        op0=mybir.AluOpType.bypass,
                        op1=mybir.AluOpType.mult,
                        accum_out=scores[:, s:s + 1],
                    )

                # Mask duplicate sample slots, then softmax over the S slots.
                nc.vector.tensor_tensor(out=scores[:], in0=scores[:], in1=mask[:],
                                        op=mybir.AluOpType.add)
                negmax = sm_pool.tile([P, 1], f32, tag="negmax")
                nc.vector.tensor_reduce(out=negmax[:], in_=scores[:],
                                        axis=mybir.AxisListType.X,
                                        op=mybir.AluOpType.max, negate=True)
                probs = sm_pool.tile([P, S], f32, tag="probs")
                denom = sm_pool.tile([P, 1], f32, tag="denom")
                nc.scalar.activation(out=probs[:], in_=scores[:],
                                     func=mybir.ActivationFunctionType.Exp,
                                     bias=negmax[:, :1], scale=1.0,
                                     accum_out=denom[:, :1])
                recip = sm_pool.tile([P, 1], f32, tag="recip")
                nc.vector.reciprocal(recip[:], denom[:])
                wts = sm_pool.tile([P, S], f32, tag="wts")
                nc.vector.tensor_scalar_mul(wts[:], probs[:], recip[:, :1])

                # out[p] = sum_s wts[p, s] * etile[p, s*F:(s+1)*F]
                acc = acc_pool.tile([P, FDIM], f32, tag="acc")
                nc.vector.tensor_scalar_mul(acc[:], etile[:, 0:FDIM], wts[:, 0:1])
                for s in range(1, S):
                    nc.vector.scalar_tensor_tensor(
                        out=acc[:],
                        in0=etile[:, s * FDIM:(s + 1) * FDIM],
                        scalar=wts[:, s:s + 1],
                        in1=acc[:],
                        op0=mybir.AluOpType.mult,
                        op1=mybir.AluOpType.add,
                    )
                nc.sync.dma_start(out=out[rows, :], in_=acc[:])

    nc.compile()
    return nc


def _multi_idx_gather_works():
    """Tiny on-device self-test of the multi-index indirect gather.

    Some device/worker states corrupt multi-index ([128, S]) indirect
    gathers while [128, 1] gathers keep working. Gather known rows from a
    small table on core 0 and verify host-side. Any failure (wrong data or
    an exception) selects the safe path.
    """
    TV = 512
    try:
        nc = bacc.Bacc("TRN2", target_bir_lowering=False, debug=False,
                       num_devices=1)
        f32, i32 = mybir.dt.float32, mybir.dt.int32
        table = nc.dram_tensor("table", [TV, FDIM], f32, kind="ExternalInput").ap()
        tidx = nc.dram_tensor("tidx", [P, S], i32, kind="ExternalInput").ap()
        tout = nc.dram_tensor("tout", [P, S * FDIM], f32, kind="ExternalOutput").ap()
        with tile.TileContext(nc) as tc:
            with tc.tile_pool(name="sb", bufs=1) as pool:
                it = pool.tile([P, S], i32)
                nc.sync.dma_start(out=it[:], in_=tidx)
                gt = pool.tile([P, S * FDIM], f32)
                nc.gpsimd.indirect_dma_start(
                    out=gt[:].rearrange("p (s f) -> p s f", s=S),
                    out_offset=None, in_=table,
                    in_offset=bass.IndirectOffsetOnAxis(ap=it[:, :], axis=0))
                nc.sync.dma_start(out=tout, in_=gt[:])
        nc.compile()
        rng = np.random.default_rng(0)
        tab = rng.standard_normal((TV, FDIM)).astype(np.float32)
        idx = rng.integers(0, TV, (P, S)).astype(np.int32)
        res = bass_utils.run_bass_kernel_spmd(
            nc, [{"table": tab, "tidx": idx}], core_ids=[0])
        got = res.results[0]["tout"]
        return np.array_equal(got, tab[idx].reshape(P, S * FDIM))
    except Exception:
        return False


def _prep_host(nodes, unique_ids, neigh_idx):
    nodes = np.asarray(nodes).astype(np.int32)
    unique_ids = np.asarray(unique_ids).astype(np.int32)
    neigh_idx = np.asarray(neigh_idx).astype(np.int32)

    # Row ids into the feature table for every (node, sample) pair.
    neigh_rows = unique_ids[neigh_idx]                      # [N, S] int32

    # Duplicate columns within a row appear once in the reference softmax:
    # mask out (additively) every repeat of an earlier column in the row.
    eq = neigh_idx[:, :, None] == neigh_idx[:, None, :]     # [N, S, S]
    earlier = np.tril(np.ones((S, S), dtype=bool), -1)      # t < s
    dup = (eq & earlier[None]).any(axis=2)                  # [N, S]
    dup_mask = np.where(dup, NEG, np.float32(0.0)).astype(np.float32)

    return nodes.reshape(N, 1), neigh_rows, dup_mask


def _make_in_maps(features, nodes, unique_ids, neigh_idx):
    features = np.ascontiguousarray(np.asarray(features), dtype=np.float32)
    node_rows, neigh_rows, dup_mask = _prep_host(nodes, unique_ids, neigh_idx)
    in_maps = []
    for c in range(NCORES):
        rows = slice(c * NPC, (c + 1) * NPC)
        in_maps.append({
            "features": features,
            "node_rows": np.ascontiguousarray(node_rows[rows]),
            "neigh_rows": np.ascontiguousarray(neigh_rows[rows]),
            "dup_mask": np.ascontiguousarray(dup_mask[rows]),
        })
    return in_maps


def _run(in_maps, **kwargs):
    if "nc" not in _CACHE:
        fast = _multi_idx_gather_works()
        _CACHE["fast"] = fast
        _CACHE["nc"] = _build_nc(multi_idx_gather=fast)
    res = bass_utils.run_bass_kernel_spmd(
        _CACHE["nc"], in_maps, core_ids=list(range(NCORES)), **kwargs)
    out = np.concatenate([res.results[c]["out"] for c in range(NCORES)], axis=0)
    return out, res


def kernel(features, nodes, unique_ids, neigh_idx):
    in_maps = _make_in_maps(features, nodes, unique_ids, neigh_idx)
    out, _ = _run(in_maps)
    return out
